# revision 19
# baseline (speedup 1.0000x reference)
"""Multi-head attention (B=4, S=2048, D=1024, H=16, HD=64) on 8 trn2 NeuronCores.

Sharding: tensor-parallel by heads. Each core owns 2 heads = 128 columns of
Wq/Wk/Wv (and 128 rows of Wo). Host pre-transposes hidden -> hT [D, B*S] (bf16);
host sums the 8 partial outputs (row-parallel out-projection) and adds bo.

Per-core dataflow (per batch b, head h):
  QT/KT [128, S]  = Wsl.T @ hT          (Wsl as stationary operand, bf16)
  V_aug [S, .]    = hT_chunk.T @ Wv_sl  (+bias, ones column at slot HD)
  scoresT [k,q]   = KT_chunk.T @ QT     (contraction 64; the two heads at base
                                         partitions 0/64 pack into disjoint PE
                                         row groups; one [128,1024] psum)
  expT bf16       = exp(scoresT / 8)    (ScalarE, scale folded into activation)
  ctxT_aug [65,q] = V_aug.T @ expT      (row 64 rides along = softmax sums)
  normalize       = DVE reciprocal + gpsimd partition_broadcast + DVE multiply
  out_partial     = ctxT_chunk.T @ Wo_sl -> DVE copy -> DMA

Scheduling: ScalarE exp is the near-critical engine (256 ACTIVATEs of
FD=1024 = ~285us busy vs ~317us of PE matmul issue).  Emission order is
both the Tile scheduler's priority and the source of dependency edges,
so the program is emitted as one rate-paced stream: per key-chunk slot,
one scores pair (feeding ScalarE), then non-scores work (projection
chains / PV quarters / normalize / out-projection) popped from a
deadline-ordered queue at a rate that spreads it evenly across the
remaining slots.  This keeps the exp stream dense instead of letting
multi-microsecond PE bursts starve ScalarE.  A dummy-matmul warmup
stream runs during the initial hT DMA so the PE is HAM-warm (2.4 GHz)
when the first projection chains issue.  PV accumulation groups are
closed per-matmul (stop=True + skip_group_check) so walrus codegen is
robust to interleaving; exp tiles are pooled 26 deep so next-window
ACTIVATEs never overwrite unconsumed tiles.
"""

import numpy as np

B, S, D, H = 4, 2048, 1024, 16
HD = D // H          # 64
NCORES = 8
HPC = H // NCORES    # heads per core = 2
CW = HPC * HD        # per-core width of Q/K/V = 128
T = B * S            # 8192 tokens
P = 128
DC = D // P          # 8 d-chunks
TB = S // 512        # 4 token blocks of 512 per batch
TC = S // P          # 16 token chunks of 128 per batch
KC = S // P          # 16 key chunks of 128
QB = S // 512        # 4 query blocks of 512

_cached = {}


def _build():
    import concourse.bass as bass
    import concourse.mybir as mybir
    import concourse.tile as tile
    from concourse import bacc

    f32 = mybir.dt.float32
    bf16 = mybir.dt.bfloat16
    nc = bacc.Bacc(
        "TRN2", target_bir_lowering=False, debug=False,
        enable_asserts=False, num_devices=NCORES,
    )

    hT = nc.dram_tensor("hT", [D, T], bf16, kind="ExternalInput").ap()
    wq = nc.dram_tensor("wq", [D, CW], bf16, kind="ExternalInput").ap()
    wk = nc.dram_tensor("wk", [D, CW], bf16, kind="ExternalInput").ap()
    wv = nc.dram_tensor("wv", [D, CW], bf16, kind="ExternalInput").ap()
    wo = nc.dram_tensor("wo", [CW, D], bf16, kind="ExternalInput").ap()
    bqd = nc.dram_tensor("bq", [CW], f32, kind="ExternalInput").ap()
    bkd = nc.dram_tensor("bk", [CW], f32, kind="ExternalInput").ap()
    bvd = nc.dram_tensor("bv", [CW], f32, kind="ExternalInput").ap()
    out = nc.dram_tensor("out", [T, D], f32, kind="ExternalOutput").ap()

    Exp = mybir.ActivationFunctionType.Exp
    mult = mybir.AluOpType.mult

    with tile.TileContext(nc) as tc:
        with (
            tc.tile_pool(name="const", bufs=1) as cpool,
            tc.tile_pool(name="ht", bufs=2) as htpool,
            tc.tile_pool(name="qkv", bufs=2) as qkvpool,
            tc.tile_pool(name="expp", bufs=26) as exppool,
            tc.tile_pool(name="ctx", bufs=2) as ctxpool,
            tc.tile_pool(name="outp", bufs=3) as outpool,
            tc.tile_pool(name="small", bufs=2) as smallpool,
            tc.tile_pool(name="mm", bufs=2, space="PSUM") as pmm,
            tc.tile_pool(name="scores", bufs=2, space="PSUM") as pscore,
            tc.tile_pool(name="acc", bufs=2, space="PSUM") as pacc,
        ):
            # ---- constants / weights (loaded once) ----
            wq_sb = cpool.tile([P, DC, CW], bf16, tag="wq")
            wk_sb = cpool.tile([P, DC, CW], bf16, tag="wk")
            wv_sb = cpool.tile([P, DC, CW], bf16, tag="wv")
            wo_sb = cpool.tile([P, D], bf16, tag="wo")

            # Warmup tile: DVE memset, then dummy matmuls stream it through
            # the PE while the first hT DMA lands, so HAM un-throttles
            # (1.2 -> 2.4 GHz) before the real projection chains start.
            warm = cpool.tile([P, 512], bf16, tag="warm")
            nc.vector.memset(warm[:], 0.0)

            def emit_warmup(n):
                for _ in range(n):
                    ps_w = pmm.tile([P, 512], f32, tag="mm", name="ps_w")
                    nc.tensor.matmul(ps_w[:], warm[:, 0:P], warm[:],
                                     start=True, stop=True)

            # DMA trigger order matters: triggers serialize on the Sync
            # engine (~0.6us each), so the first-ACT critical path (wk, wq,
            # hT block 0) goes first; biases/wv/wo are needed later.
            nc.sync.dma_start(wk_sb[:], wk.rearrange("(o p) c -> p o c", p=P))
            nc.sync.dma_start(wq_sb[:], wq.rearrange("(o p) c -> p o c", p=P))

            bq_sb = cpool.tile([P, 1], f32, tag="bq")
            bk_sb = cpool.tile([P, 1], f32, tag="bk")
            bv_row = cpool.tile([1, CW], f32, tag="bvr")
            bv_bf = cpool.tile([1, CW], bf16, tag="bvbf")

            def emit_bias_loads():
                nc.sync.dma_start(bk_sb[:], bkd.unsqueeze(1))
                nc.sync.dma_start(bq_sb[:], bqd.unsqueeze(1))
                nc.sync.dma_start(bv_row[:], bvd.unsqueeze(0))
                nc.vector.tensor_copy(bv_bf[:], bv_row[:])

            ones_bf = cpool.tile([1, P], bf16, tag="onesbf")
            nc.vector.memset(ones_bf[:], 1.0)
            bv_bc = cpool.tile([P, CW], f32, tag="bvbc")

            def emit_bv_broadcast():
                # bv broadcast across partitions: bv_bc[p, c] = bv[c]
                ps_bv = pmm.tile([P, 512], f32, tag="mm")
                nc.tensor.matmul(ps_bv[:, :CW], ones_bf[0:1, :],
                                 bv_bf[0:1, :], start=True, stop=True)
                nc.vector.tensor_copy(bv_bc[:], ps_bv[:, :CW])

            def emit_load(b):
                """Allocate per-batch tiles and start the hT DMA. Batch 0's
                first token block is split into per-2dc transfers so the first
                projection chain starts as soon as dc0/1 land (DMA triggers
                serialize on the Sync engine at ~0.6us each, so fewer, larger
                transfers win elsewhere)."""
                ht_b = htpool.tile([P, DC, S], bf16, tag="ht", name="ht_b")
                for tb in range(TB):
                    tsl = slice(b * S + tb * 512, b * S + (tb + 1) * 512)
                    src = hT[:, tsl].rearrange("(o p) t -> p o t", p=P)
                    if b == 0 and tb == 0:
                        for dc in range(0, DC, 2):
                            nc.sync.dma_start(
                                ht_b[:, dc:dc + 2, 0:512],
                                src[:, dc:dc + 2, :])
                    else:
                        nc.sync.dma_start(
                            ht_b[:, :, tb * 512:(tb + 1) * 512], src)
                qt = qkvpool.tile([P, S], bf16, tag="qt", name="qt")
                kt = qkvpool.tile([P, S], bf16, tag="kt", name="kt")
                v_aug = qkvpool.tile([P, TC, HPC, HD + 1], bf16, tag="vaug",
                                     name="v_aug")
                return ht_b, qt, kt, v_aug

            def emit_qkt_half(st, tb, dst_i, half, ps_box):
                """Half (4 d-chunks) of a 512-token-block projection chain for
                QT (dst_i=0) or KT (dst_i=1).  Split in two queue units so a
                scores pair can slip in between; the accumulation group stays
                open across the split (stop+skip_group_check per matmul)."""
                ht_b, qt, kt, _ = st
                dst, w_sb, bias = ((qt, wq_sb, bq_sb), (kt, wk_sb, bk_sb))[dst_i]
                if half == 0:
                    ps_box[0] = pmm.tile([P, 512], f32, tag="mm", name="ps_p")
                ps = ps_box[0]
                for dc in range(half * 4, half * 4 + 4):
                    nc.tensor.matmul(
                        ps[:], w_sb[:, dc, :],
                        ht_b[:, dc, tb * 512:(tb + 1) * 512],
                        start=(dc == 0), stop=True, skip_group_check=True)
                if half == 1:
                    nc.vector.tensor_scalar_add(
                        dst[:, tb * 512:(tb + 1) * 512], ps[:], bias[:, 0:1])

            def emit_v_chain(st, tcj):
                """One 128-token-chunk projection chain for V_aug."""
                ht_b, _, _, v_aug = st
                if tcj == 0:
                    nc.vector.memset(v_aug[:, :, :, HD:HD + 1], 1.0)
                ps = pmm.tile([P, 512], f32, tag="mm", name="ps_v")
                for dc in range(DC):
                    nc.tensor.matmul(
                        ps[:, :CW], ht_b[:, dc, tcj * P:(tcj + 1) * P],
                        wv_sb[:, dc, :],
                        start=(dc == 0), stop=(dc == DC - 1))
                # psum [128tok, (h, hd)] + bias -> v_aug[tok, tcj, h, 0:HD]
                nc.vector.tensor_add(
                    v_aug[:, tcj, :, 0:HD], ps[:, :CW], bv_bc[:, :CW])

            def emit_scores_kc(st, qb, kc, exps):
                """Scores+exp for one key chunk of a 512-wide query block.
                The two heads' K=64 score matmuls pack into disjoint PE row
                groups and share one [128,1024] psum so exp runs at FD=1024."""
                _, qt, kt, _ = st
                qsl = slice(qb * 512, (qb + 1) * 512)
                ps_s = pscore.tile([P, 1024], f32, tag="sc", name="ps_s")
                for h in range(HPC):
                    hs = slice(h * HD, (h + 1) * HD)
                    nc.tensor.matmul(
                        ps_s[:, h * 512:(h + 1) * 512],
                        kt[hs, kc * P:(kc + 1) * P],
                        qt[hs, qsl], start=True, stop=True)
                ex = exppool.tile([P, 1024], bf16, tag="expT", name="ex")
                nc.scalar.activation(ex[:], ps_s[:], Exp, scale=1.0 / 8.0)
                exps.append(ex)

            def emit_pv_unit(st, pcs, h, kcs, exps, first):
                """PV accumulation over a kc range for one head."""
                _, _, _, v_aug = st
                if first:
                    pcs[h] = pacc.tile([P, 512], f32, tag="ctx",
                                       name="ps_ctx")
                for kc in kcs:
                    nc.tensor.matmul(
                        pcs[h][0:HD + 1, :], v_aug[:, kc, h, :],
                        exps[kc][:, h * 512:(h + 1) * 512],
                        start=(kc == 0), stop=True,
                        skip_group_check=True)

            ones_hd = cpool.tile([1, HD], bf16, tag="oneshd")
            nc.vector.memset(ones_hd[:], 1.0)

            def emit_norm_head(pcs, ctxt, qb, h, pe_bcast=False):
                """Softmax normalize one head: reciprocal of the ridden-along
                sums row, partition-broadcast, fused DVE multiply.  The
                broadcast runs on gpsimd mid-stream (PE is the scarce engine
                there) but as a PE matmul in the epilogue (gpsimd's 1us
                latency would sit on the critical path while PE idles)."""
                qsl = slice(qb * 512, (qb + 1) * 512)
                hs = slice(h * HD, (h + 1) * HD)
                ps_ctx = pcs[h]
                sums = smallpool.tile([1, 512], f32, tag="sums",
                                      bufs=4, name="sums")
                nc.vector.tensor_copy(sums[:], ps_ctx[HD:HD + 1, :])
                recip = smallpool.tile([1, 512], f32, tag="recip",
                                       bufs=4, name="recip")
                nc.vector.reciprocal_approx_fast(recip[:], sums[:])
                if pe_bcast:
                    recip_bf = smallpool.tile([1, 512], bf16, tag="recipbf",
                                              bufs=4, name="recipbf")
                    nc.vector.tensor_copy(recip_bf[:], recip[:])
                    ps_r = pscore.tile([P, 1024], f32, tag="sc", name="ps_r")
                    nc.tensor.matmul(ps_r[0:HD, 0:512], ones_hd[0:1, :],
                                     recip_bf[0:1, :], start=True, stop=True)
                    nc.vector.tensor_tensor(ctxt[hs, qsl], ps_ctx[0:HD, :],
                                            ps_r[0:HD, 0:512], mult)
                else:
                    rbc = smallpool.tile([HD, 512], f32, tag="rbc",
                                         bufs=4, name="rbc")
                    nc.gpsimd.partition_broadcast(rbc[:], recip[:])
                    nc.vector.tensor_tensor(ctxt[hs, qsl], ps_ctx[0:HD, :],
                                            rbc[:], mult)

            def emit_outproj_tcj(ctxt, b, tcj, scalar_copy=False):
                """Out-projection for one 128-token chunk."""
                tsl = slice(b * S + tcj * P, b * S + (tcj + 1) * P)
                out_sb = outpool.tile([P, D], f32, tag="out", name="out_sb")
                for half in range(2):
                    ps_o = pmm.tile([P, 512], f32, tag="mm", name="ps_o")
                    nc.tensor.matmul(
                        ps_o[:], ctxt[:, tcj * P:(tcj + 1) * P],
                        wo_sb[:, half * 512:(half + 1) * 512],
                        start=True, stop=True)
                    if scalar_copy and half == 1:
                        nc.scalar.copy(
                            out_sb[:, half * 512:(half + 1) * 512], ps_o[:])
                    else:
                        nc.vector.tensor_copy(
                            out_sb[:, half * 512:(half + 1) * 512], ps_o[:])
                nc.sync.dma_start(out[tsl, :], out_sb[:])

            # ---- rate-paced software pipeline ----
            # One slot per (b, qb, kc) scores pair.  Non-scores work lives in
            # a deadline-ordered queue; each slot emits the scores pair, then
            # pops queue work: everything past its deadline, then enough to
            # keep the queue draining evenly over the remaining slots.
            NSLOT = B * QB * KC

            def slot_of(b, qb, kc):
                return (b * QB + qb) * KC + kc

            work = []   # list of (deadline_slot, cost_ns, fn, ready_slot)

            def push(dl, cost, fn, ready=0):
                work.append((max(dl, ready), cost, fn, ready))

            state = [None] * B   # per-batch (ht, qt, kt, v_aug)
            ctxts = [None] * B

            def enqueue_batch_chains(b, dl0):
                """Projection chains for batch b, with deadlines relative to
                its first scores slot."""
                st = state[b]
                for tb in range(TB):
                    box = [None]
                    for half in range(2):
                        push(max(dl0 + 4 * tb - 3 + half, 0), 915,
                             lambda st=st, tb=tb, half=half, box=box:
                             emit_qkt_half(st, tb, 1, half, box))
                for qb in range(QB):
                    box = [None]
                    for half in range(2):
                        push(max(dl0 + 16 * qb - 3 + half, 0), 915,
                             lambda st=st, qb=qb, half=half, box=box:
                             emit_qkt_half(st, qb, 0, half, box))
                # V(tcj) feeds PV(b, qb0, j=tcj//4) whose deadline is
                # dl0 + 16 + 2*(tcj//4) + 4; stay strictly ahead of it,
                # one chain per slot to avoid bursts.
                for tcj in range(TC):
                    push(dl0 + 16 + 2 * (tcj // 4) + (tcj % 4), 750,
                         lambda st=st, tcj=tcj: emit_v_chain(st, tcj))

            def enqueue_attention_tail(b, qb, exps, last):
                """PV + normalize + out-projection of (b, qb), deadlined into
                the following window (or drained at the end for the last qb)."""
                st = state[b]
                ctxt = ctxts[b]
                pcs = [None, None]
                base = slot_of(b, qb, 0) + KC     # next window's first slot
                for j in range(4):
                    for h in range(HPC):
                        if last:
                            # ready only once exp tiles 4j..4j+3 are emitted
                            rdy = slot_of(b, qb, 0) + 4 * j + 4
                            dl = min(rdy + h, NSLOT)
                        else:
                            rdy = 0
                            dl = base + 2 * j + h + 4
                        push(dl, 964,
                             lambda st=st, pcs=pcs, h=h, j=j, exps=exps:
                             emit_pv_unit(st, pcs, h, range(4 * j, 4 * j + 4),
                                          exps, j == 0),
                             ready=rdy)
                for h in range(HPC):
                    dl = NSLOT + 1 if last else base + 12
                    push(dl, 0,
                         lambda pcs=pcs, ctxt=ctxt, qb=qb, h=h, last=last:
                         emit_norm_head(pcs, ctxt, qb, h, pe_bcast=last),
                         ready=NSLOT if last else 0)
                for i, tcj in enumerate(range(qb * 4, qb * 4 + 4)):
                    dl = NSLOT + 2 + i if last else base + 13 + i
                    push(dl, 532,
                         lambda ctxt=ctxt, b=b, tcj=tcj, last=last:
                         emit_outproj_tcj(ctxt, b, tcj, scalar_copy=last),
                         ready=NSLOT if last else 0)

            # prologue: warm the PE during the initial DMAs, then stage
            # batch 0 and its chains.
            state[0] = emit_load(0)
            emit_bias_loads()
            nc.sync.dma_start(wv_sb[:], wv.rearrange("(o p) c -> p o c", p=P))
            nc.sync.dma_start(wo_sb[:], wo)
            emit_warmup(6)
            emit_bv_broadcast()
            ctxts[0] = ctxpool.tile([P, S], bf16, tag="ctxt", name="ctxt")
            enqueue_batch_chains(0, 0)

            emitted_other = 0.0
            target_other = 0.0

            for b in range(B):
                for qb in range(QB):
                    exps = []
                    last = (b == B - 1 and qb == QB - 1)
                    if b + 1 < B and qb == 0:
                        state[b + 1] = emit_load(b + 1)
                        ctxts[b + 1] = ctxpool.tile([P, S], bf16, tag="ctxt",
                                                    name="ctxt")
                        enqueue_batch_chains(b + 1, slot_of(b + 1, 0, 0))
                    if last:
                        # enqueue the tail up front: its PV units interleave
                        # into this window as the exp tiles appear (the
                        # lambdas capture the still-filling exps list).
                        enqueue_attention_tail(b, qb, exps, last=True)
                    for kc in range(KC):
                        s = slot_of(b, qb, kc)
                        # deadline-forced work first (keeps read-after-write
                        # emission order for everything the pair consumes)
                        work.sort(key=lambda w: w[0])
                        while work and work[0][0] <= s:
                            _, c, fn, _ = work.pop(0)
                            fn()
                            emitted_other += c
                        emit_scores_kc(state[b], qb, kc, exps)
                        # pace the queue: spread remaining work over the
                        # remaining slots before its last deadline, capped so
                        # a burst never starves ScalarE; additionally smooth
                        # any deadline cluster looming within 2 slots.
                        if work:
                            rem = sum(c for _, c, _, _ in work)
                            dmax = max(dl for dl, _, _, _ in work)
                            rate = rem / max(dmax - s, 1)
                            target_other += min(rate, 1900.0)
                            while (work and emitted_other < target_other
                                   and work[0][3] <= s):
                                _, c, fn, _ = work.pop(0)
                                fn()
                                emitted_other += c
                            due = sum(c for dl, c, _, _ in work
                                      if dl <= s + 2)
                            while due > 2600 and work and work[0][3] <= s:
                                dl, c, fn, _ = work.pop(0)
                                fn()
                                emitted_other += c
                                due -= c
                    if not last:
                        enqueue_attention_tail(b, qb, exps, last=False)

            # epilogue: drain whatever remains (last qb's PV tail, normalize,
            # out-projection).
            work.sort(key=lambda w: w[0])
            for _, _, fn, _ in work:
                fn()
            work.clear()

    nc.compile()
    return nc


def _get_nc():
    if "nc" not in _cached:
        _cached["nc"] = _build()
    return _cached["nc"]


def kernel(hidden_states, attention_mask, Wq, bq, Wk, bk, Wv, bv, Wo, bo):
    res = kernel_run(hidden_states, Wq, bq, Wk, bk, Wv, bv, Wo)
    total = np.zeros((T, D), np.float32)
    for r in res.results:
        total += r["out"]
    total += np.asarray(bo, np.float32)[None, :]
    return total.reshape(B, S, D)


def kernel_run(hidden_states, Wq, bq, Wk, bk, Wv, bv, Wo, **run_kwargs):
    import ml_dtypes
    from concourse.bass_utils import run_bass_kernel_spmd

    nc = _get_nc()
    bf = ml_dtypes.bfloat16

    hT = np.ascontiguousarray(
        np.asarray(hidden_states, dtype=np.float32).reshape(T, D).T).astype(bf)
    Wq = np.asarray(Wq, np.float32).astype(bf)
    Wk = np.asarray(Wk, np.float32).astype(bf)
    Wv = np.asarray(Wv, np.float32).astype(bf)
    Wo = np.asarray(Wo, np.float32).astype(bf)
    bq = np.asarray(bq, np.float32); bk = np.asarray(bk, np.float32)
    bv = np.asarray(bv, np.float32)

    in_maps = []
    for c in range(NCORES):
        cs = slice(c * CW, (c + 1) * CW)
        in_maps.append({
            "hT": hT,
            "wq": np.ascontiguousarray(Wq[:, cs]),
            "wk": np.ascontiguousarray(Wk[:, cs]),
            "wv": np.ascontiguousarray(Wv[:, cs]),
            "wo": np.ascontiguousarray(Wo[cs, :]),
            "bq": np.ascontiguousarray(bq[cs]),
            "bk": np.ascontiguousarray(bk[cs]),
            "bv": np.ascontiguousarray(bv[cs]),
        })

    return run_bass_kernel_spmd(
        nc, in_maps, core_ids=list(range(NCORES)), **run_kwargs)
